# revision 5
# baseline (speedup 1.0000x reference)
"""Trainium2 Bass kernel for nn_Attn_55611236548746.

Attention pooling:
    energies[b,t] = enc[b,t,:]@w_e + hid_flat[b,:]@w_h + bias
    p = renorm(mask * softmax(energies * mask))
    out[b,:]     = sum_t p[b,t] * enc[b,t,:]

Sharding: data-parallel over B (32 batches -> 4 per core on 8 cores);
attn weights replicated.

Algebra: the hidden projection + bias are constant over t within a
batch, so they cancel in the softmax renorm (exp(en+c)/sum exp(en+c) ==
exp(en)/sum exp(en)); the inner mask multiply only changes masked-out
positions, which the outer mask zeroes anyway. Hence
    p_t = mask_t * exp(en_t) / sum_t mask_t * exp(en_t),
    en_t = enc[t,:] @ w_e
and hidden/attn_b never enter the kernel. No max subtraction needed
(|en| < ~8 for this data scale; reference computes the same way in f32).

Per-core schedule (memory-bound; HBM floor = 32MB / ~425GB/s ~ 75us):
  - enc streams as 16 quarter-tiles (128t x 4j x 1024e) via gpsimd
    SWDGE casting DMA f32->f32r (16KB contiguous descriptors; the
    verifier requires matmul f32r inputs to be DMA-rounded, so HWDGE
    f32 + bitcast is not allowed). bufs=10 keeps 2.5 batches in
    flight. DVE reads the same tiles as bitcast f32.
  - energies: DVE scalar_tensor_tensor (mult + row-sum accum) per
    128x1024 tile -> en column (1.22us each; DVE is co-critical).
  - per quarter: exp on ScalarE -> DVE mask-mult (+us accum) ->
    ScalarE f32r cast -> 8 PE pool matmuls, so PE trails DVE by one
    quarter and the post-DMA tail is ~5us.
  - weighted pool: PE matmuls contracting over t (u column as lhsT,
    f32r full rate), accumulated in fp32 PSUM; final 1/sum scale on
    ScalarE via activation(scale=reciprocal).
"""

import numpy as np

N_CORES = 8
B, T, E = 32, 2048, 1024
LD, HD = 2, 1024          # hidden: (LD, B, HD)
DEC = LD * HD             # 2048 = flattened-hidden width
BP = B // N_CORES         # 4 batches per core
TB = T // 128             # 16 t-blocks of 128
NQ = 4                    # quarters per batch
TQ = TB // NQ             # t-blocks per quarter

_nc_cache = {}


def _build(reps=1, body_mult=1, mode="full"):
    from contextlib import ExitStack

    import concourse.bacc as bacc
    import concourse.tile as tile
    from concourse import mybir
    from concourse._compat import with_exitstack
    from concourse.alu_op_type import AluOpType

    f32 = mybir.dt.float32
    f32r = mybir.dt.float32r
    MUL, ADD = AluOpType.mult, AluOpType.add
    EXP = mybir.ActivationFunctionType.Exp
    COPY = mybir.ActivationFunctionType.Copy

    nc = bacc.Bacc("TRN2", target_bir_lowering=False, debug=False,
                   num_devices=N_CORES)
    enc = nc.dram_tensor("enc", [BP, T, E], f32, kind="ExternalInput").ap()
    hid = nc.dram_tensor("hid", [LD, BP, HD], f32, kind="ExternalInput").ap()
    msk = nc.dram_tensor("msk", [BP, T], f32, kind="ExternalInput").ap()
    w = nc.dram_tensor("w", [DEC + E], f32, kind="ExternalInput").ap()
    bia = nc.dram_tensor("bia", [1], f32, kind="ExternalInput").ap()
    out = nc.dram_tensor("out", [BP, E], f32, kind="ExternalOutput").ap()
    del hid, bia  # cancel in the softmax renorm (see module docstring)

    @with_exitstack
    def body(ctx, tc):
        consts = ctx.enter_context(tc.tile_pool(name="consts", bufs=1))
        encp = ctx.enter_context(tc.tile_pool(name="encp", bufs=10))
        scrp = ctx.enter_context(tc.tile_pool(name="scrp", bufs=2))
        small = ctx.enter_context(tc.tile_pool(name="small", bufs=3))
        outp = ctx.enter_context(tc.tile_pool(name="outp", bufs=2))
        pso = ctx.enter_context(tc.tile_pool(name="pso", bufs=2, space="PSUM"))
        pst = ctx.enter_context(tc.tile_pool(name="pst", bufs=2, space="PSUM"))

        # consts on the scalar HWDGE queue. Mask first: its 64B-per-
        # partition descriptors must beat the enc flood to the DMA
        # engines (a late mask stalls the whole u/pool cascade).
        mask_sb = consts.tile([128, BP, TB], f32)
        for b in range(BP):
            nc.scalar.dma_start(out=mask_sb[:, b, :],
                                in_=msk[b].rearrange("(p j) -> p j", p=128))
        # w_e: one 4KB descriptor to partition 0, then a K=1 PE
        # outer-product broadcast (ones row x w row) to all partitions.
        w_row = consts.tile([1, E], f32)
        nc.scalar.dma_start(out=w_row, in_=w[None, DEC:DEC + E])
        ones_row = consts.tile([1, 128], f32)
        nc.vector.memset(ones_row, 1.0)
        ones_col = consts.tile([128, 1], f32)
        nc.vector.memset(ones_col, 1.0)
        w_bc = consts.tile([128, E], f32)
        psw = ctx.enter_context(tc.tile_pool(name="psw", bufs=1, space="PSUM"))
        for c in range(2):
            sl = slice(512 * c, 512 * (c + 1))
            wp = psw.tile([128, 512], f32)
            nc.tensor.matmul(wp, ones_row, w_row[:, sl], start=True, stop=True)
            nc.scalar.copy(out=w_bc[:, sl], in_=wp)

        # enc quarter loads: all on the sync HWDGE queue, issued upfront
        # in stream order; encp bufs gate the tail ones naturally.
        qt = []
        for b in range(BP):
            encb = enc[b].rearrange("(p j) e -> p j e", p=128)
            for q in range(NQ):
                t_ = encp.tile([128, TQ, E], f32r)
                nc.gpsimd.dma_start(out=t_,
                                    in_=encb[:, TQ * q:TQ * (q + 1), :])
                qt.append(t_)

        def main_loop():
            for b in range(BP):
                en = small.tile([128, TB], f32)
                u0 = small.tile([128, TB], f32)
                u = small.tile([128, TB], f32)
                ur = small.tile([128, TB], f32r)
                us4 = small.tile([128, NQ], f32)
                po = pso.tile([1, E], f32)
                tot = pst.tile([1, 1], f32)

                for q in range(NQ):
                    enc_q = qt[b * NQ + q]
                    sl_t = slice(TQ * q, TQ * (q + 1))

                    if mode != "dma":
                        # energies: en[:, 4q+i] = enc_tile @ w_e
                        for i in range(TQ):
                            s = scrp.tile([128, E], f32)
                            nc.vector.scalar_tensor_tensor(
                                out=s, in0=enc_q[:, i, :].bitcast(f32),
                                scalar=0.0,
                                in1=w_bc, op0=ADD, op1=MUL,
                                accum_out=en[:, TQ * q + i:TQ * q + i + 1])
                    else:
                        sink = small.tile([1, 4], f32)
                        nc.vector.tensor_copy(
                            sink, enc_q[0:1, 0, 0:4].bitcast(f32))
                        continue

                    if mode == "dve":
                        continue

                    # u = mask * exp(en); us4[:, q] = row-sum of u quarter
                    nc.scalar.activation(out=u0[:, sl_t], in_=en[:, sl_t],
                                         func=EXP)
                    nc.vector.scalar_tensor_tensor(
                        out=u[:, sl_t], in0=u0[:, sl_t], scalar=0.0,
                        in1=mask_sb[:, b, sl_t], op0=ADD, op1=MUL,
                        accum_out=us4[:, q:q + 1])
                    nc.scalar.copy(out=ur[:, sl_t], in_=u[:, sl_t])

                    # weighted pool for this quarter (PSUM-accumulating)
                    for half in range(2):
                        sl_e = slice(half * 512, (half + 1) * 512)
                        for i in range(TQ):
                            nc.tensor.matmul(
                                po[:, sl_e], ur[:, TQ * q + i:TQ * q + i + 1],
                                enc_q[:, i, sl_e],
                                start=(q == 0 and i == 0),
                                stop=(q == NQ - 1 and i == TQ - 1))

                if mode in ("dma", "dve"):
                    continue

                us1 = small.tile([128, 1], f32)
                nc.vector.tensor_reduce(out=us1, in_=us4,
                                        axis=mybir.AxisListType.X, op=ADD)
                nc.tensor.matmul(tot, us1, ones_col, start=True, stop=True)
                rt = small.tile([1, 1], f32)
                nc.vector.reciprocal(out=rt, in_=tot)
                ob = outp.tile([1, E], f32)
                nc.scalar.activation(out=ob, in_=po, func=COPY, scale=rt)
                nc.scalar.dma_start(out=out[b], in_=ob)

        if reps == 1:
            for _ in range(body_mult):
                main_loop()
        else:
            with tc.For_i(0, reps, 1):
                for _ in range(body_mult):
                    main_loop()

    with tile.TileContext(nc) as tc:
        body(tc)
    nc.compile()
    return nc


def _get_nc(reps=1, body_mult=1, mode="full"):
    key = (reps, body_mult, mode)
    if key not in _nc_cache:
        _nc_cache[key] = _build(reps, body_mult, mode)
    return _nc_cache[key]


def _run(hidden, encoder_outputs, mask, attn_w, attn_b, trace=False,
         trace_kwargs=None, reps=1, body_mult=1, mode="full"):
    from concourse.bass_utils import run_bass_kernel_spmd

    nc = _get_nc(reps, body_mult, mode)
    in_maps = []
    for i in range(N_CORES):
        lo = i * BP
        in_maps.append({
            "enc": np.ascontiguousarray(encoder_outputs[lo:lo + BP]),
            "hid": np.ascontiguousarray(hidden[:, lo:lo + BP, :]),
            "msk": np.ascontiguousarray(mask[lo:lo + BP]),
            "w": np.ascontiguousarray(attn_w),
            "bia": np.ascontiguousarray(attn_b),
        })
    res = run_bass_kernel_spmd(nc, in_maps, list(range(N_CORES)),
                               trace=trace, **(trace_kwargs or {}))
    full = np.concatenate([res.results[i]["out"] for i in range(N_CORES)],
                          axis=0)
    return full, res


def kernel(hidden, encoder_outputs, mask, attn_w, attn_b):
    hidden = np.asarray(hidden, dtype=np.float32)
    encoder_outputs = np.asarray(encoder_outputs, dtype=np.float32)
    mask = np.asarray(mask, dtype=np.float32)
    attn_w = np.asarray(attn_w, dtype=np.float32)
    attn_b = np.asarray(attn_b, dtype=np.float32)
    full, _ = _run(hidden, encoder_outputs, mask, attn_w, attn_b)
    return full


# revision 6
# speedup vs baseline: 1.0934x; 1.0934x over previous
"""Trainium2 Bass kernel for nn_Attn_55611236548746.

Attention pooling:
    energies[b,t] = enc[b,t,:]@w_e + hid_flat[b,:]@w_h + bias
    p = renorm(mask * softmax(energies * mask))
    out[b,:]     = sum_t p[b,t] * enc[b,t,:]

Sharding: data-parallel over B (32 batches -> 4 per core on 8 cores);
attn weights replicated.

Algebra: the hidden projection + bias are constant over t within a
batch, so they cancel in the softmax renorm (exp(en+c)/sum exp(en+c) ==
exp(en)/sum exp(en)); the inner mask multiply only changes masked-out
positions, which the outer mask zeroes anyway. Hence
    p_t = mask_t * exp(en_t) / sum_t mask_t * exp(en_t),
    en_t = enc[t,:] @ w_e
and hidden/attn_b never enter the kernel. No max subtraction needed
(|en| < ~8 for this data scale; reference computes the same way in f32).

Per-core schedule (memory-bound; HBM floor = 32MB / ~425GB/s ~ 75us):
  - enc streams as 16 quarter-tiles (128t x 4j x 1024e) via gpsimd
    SWDGE casting DMA f32->f32r (16KB contiguous descriptors; the
    verifier requires matmul f32r inputs to be DMA-rounded, so HWDGE
    f32 + bitcast is not allowed). bufs=10 keeps 2.5 batches in
    flight. DVE reads the same tiles as bitcast f32.
  - energies: DVE scalar_tensor_tensor (mult + row-sum accum) per
    128x1024 tile -> en column (1.22us each; DVE is co-critical).
  - per quarter: exp on ScalarE -> DVE mask-mult (+us accum) ->
    ScalarE f32r cast -> 8 PE pool matmuls, so PE trails DVE by one
    quarter and the post-DMA tail is ~5us.
  - weighted pool: PE matmuls contracting over t (u column as lhsT,
    f32r full rate), accumulated in fp32 PSUM; final 1/sum scale on
    ScalarE via activation(scale=reciprocal).
"""

import numpy as np

N_CORES = 8
B, T, E = 32, 2048, 1024
LD, HD = 2, 1024          # hidden: (LD, B, HD)
DEC = LD * HD             # 2048 = flattened-hidden width
BP = B // N_CORES         # 4 batches per core
TB = T // 128             # 16 t-blocks of 128
NQ = 4                    # quarters per batch
TQ = TB // NQ             # t-blocks per quarter

_nc_cache = {}


def _build(reps=1, body_mult=1, mode="full"):
    from contextlib import ExitStack

    import concourse.bacc as bacc
    import concourse.tile as tile
    from concourse import mybir
    from concourse._compat import with_exitstack
    from concourse.alu_op_type import AluOpType

    f32 = mybir.dt.float32
    f32r = mybir.dt.float32r
    MUL, ADD = AluOpType.mult, AluOpType.add
    EXP = mybir.ActivationFunctionType.Exp
    COPY = mybir.ActivationFunctionType.Copy

    nc = bacc.Bacc("TRN2", target_bir_lowering=False, debug=False,
                   num_devices=N_CORES)
    enc = nc.dram_tensor("enc", [BP, T, E], f32, kind="ExternalInput").ap()
    hid = nc.dram_tensor("hid", [LD, BP, HD], f32, kind="ExternalInput").ap()
    msk = nc.dram_tensor("msk", [BP, T], f32, kind="ExternalInput").ap()
    w = nc.dram_tensor("w", [DEC + E], f32, kind="ExternalInput").ap()
    bia = nc.dram_tensor("bia", [1], f32, kind="ExternalInput").ap()
    out = nc.dram_tensor("out", [BP, E], f32, kind="ExternalOutput").ap()
    del hid, bia  # cancel in the softmax renorm (see module docstring)

    @with_exitstack
    def body(ctx, tc):
        consts = ctx.enter_context(tc.tile_pool(name="consts", bufs=1))
        encp = ctx.enter_context(tc.tile_pool(name="encp", bufs=10))
        scrp = ctx.enter_context(tc.tile_pool(name="scrp", bufs=2))
        small = ctx.enter_context(tc.tile_pool(name="small", bufs=3))
        outp = ctx.enter_context(tc.tile_pool(name="outp", bufs=2))
        pso = ctx.enter_context(tc.tile_pool(name="pso", bufs=2, space="PSUM"))
        pst = ctx.enter_context(tc.tile_pool(name="pst", bufs=2, space="PSUM"))

        # All prerequisite loads ride the gpsimd SWDGE queue (queue FIFO
        # is the only guaranteed ordering: the enc flood starves other
        # queues' descriptors at the shared DMA engines for ~25us).
        # Order: w_row (1 descriptor), enc b0q0, then per-b mask slices
        # interleaved between the first enc quarters.
        w_row = consts.tile([1, E], f32)
        nc.gpsimd.dma_start(out=w_row, in_=w[None, DEC:DEC + E])
        ones_row = consts.tile([1, 128], f32)
        nc.vector.memset(ones_row, 1.0)
        ones_col = consts.tile([128, 1], f32)
        nc.vector.memset(ones_col, 1.0)
        mask_sb = consts.tile([128, BP, TB], f32)

        # enc quarter loads: all on the gpsimd SWDGE queue, issued
        # upfront in stream order; encp bufs gate the tail ones.
        qt = []
        for b in range(BP):
            encb = enc[b].rearrange("(p j) e -> p j e", p=128)
            for q in range(NQ):
                t_ = encp.tile([128, TQ, E], f32r)
                nc.gpsimd.dma_start(out=t_,
                                    in_=encb[:, TQ * q:TQ * (q + 1), :])
                qt.append(t_)
                if b == 0:  # mask b after enc quarter q=b of batch 0
                    nc.gpsimd.dma_start(
                        out=mask_sb[:, q, :],
                        in_=msk[q].rearrange("(p j) -> p j", p=128))

        # w_e broadcast: K=1 PE outer product (ones row x w row) into
        # PSUM, copied to SBUF on ScalarE. Ready well before the first
        # energies STT needs it.
        w_bc = consts.tile([128, E], f32)
        psw = ctx.enter_context(tc.tile_pool(name="psw", bufs=1, space="PSUM"))
        for c in range(2):
            sl = slice(512 * c, 512 * (c + 1))
            wp = psw.tile([128, 512], f32)
            nc.tensor.matmul(wp, ones_row, w_row[:, sl], start=True, stop=True)
            nc.scalar.copy(out=w_bc[:, sl], in_=wp)

        def main_loop():
            for b in range(BP):
                en = small.tile([128, TB], f32)
                u0 = small.tile([128, TB], f32)
                u = small.tile([128, TB], f32)
                ur = small.tile([128, TB], f32r)
                us4 = small.tile([128, NQ], f32)
                po = pso.tile([1, E], f32)
                tot = pst.tile([1, 1], f32)

                for q in range(NQ):
                    enc_q = qt[b * NQ + q]
                    sl_t = slice(TQ * q, TQ * (q + 1))

                    if mode != "dma":
                        # energies: en[:, 4q+i] = enc_tile @ w_e
                        for i in range(TQ):
                            s = scrp.tile([128, E], f32)
                            nc.vector.scalar_tensor_tensor(
                                out=s, in0=enc_q[:, i, :].bitcast(f32),
                                scalar=0.0,
                                in1=w_bc, op0=ADD, op1=MUL,
                                accum_out=en[:, TQ * q + i:TQ * q + i + 1])
                    else:
                        sink = small.tile([1, 4], f32)
                        nc.vector.tensor_copy(
                            sink, enc_q[0:1, 0, 0:4].bitcast(f32))
                        continue

                    if mode == "dve":
                        continue

                    # u = mask * exp(en); us4[:, q] = row-sum of u quarter
                    nc.scalar.activation(out=u0[:, sl_t], in_=en[:, sl_t],
                                         func=EXP)
                    nc.vector.scalar_tensor_tensor(
                        out=u[:, sl_t], in0=u0[:, sl_t], scalar=0.0,
                        in1=mask_sb[:, b, sl_t], op0=ADD, op1=MUL,
                        accum_out=us4[:, q:q + 1])
                    nc.scalar.copy(out=ur[:, sl_t], in_=u[:, sl_t])

                    # weighted pool for this quarter (PSUM-accumulating)
                    for half in range(2):
                        sl_e = slice(half * 512, (half + 1) * 512)
                        for i in range(TQ):
                            nc.tensor.matmul(
                                po[:, sl_e], ur[:, TQ * q + i:TQ * q + i + 1],
                                enc_q[:, i, sl_e],
                                start=(q == 0 and i == 0),
                                stop=(q == NQ - 1 and i == TQ - 1))

                if mode in ("dma", "dve"):
                    continue

                us1 = small.tile([128, 1], f32)
                nc.vector.tensor_reduce(out=us1, in_=us4,
                                        axis=mybir.AxisListType.X, op=ADD)
                nc.tensor.matmul(tot, us1, ones_col, start=True, stop=True)
                rt = small.tile([1, 1], f32)
                nc.vector.reciprocal(out=rt, in_=tot)
                ob = outp.tile([1, E], f32)
                nc.scalar.activation(out=ob, in_=po, func=COPY, scale=rt)
                nc.scalar.dma_start(out=out[b], in_=ob)

        if reps == 1:
            for _ in range(body_mult):
                main_loop()
        else:
            with tc.For_i(0, reps, 1):
                for _ in range(body_mult):
                    main_loop()

    with tile.TileContext(nc) as tc:
        body(tc)
    nc.compile()
    return nc


def _get_nc(reps=1, body_mult=1, mode="full"):
    key = (reps, body_mult, mode)
    if key not in _nc_cache:
        _nc_cache[key] = _build(reps, body_mult, mode)
    return _nc_cache[key]


def _run(hidden, encoder_outputs, mask, attn_w, attn_b, trace=False,
         trace_kwargs=None, reps=1, body_mult=1, mode="full"):
    from concourse.bass_utils import run_bass_kernel_spmd

    nc = _get_nc(reps, body_mult, mode)
    in_maps = []
    for i in range(N_CORES):
        lo = i * BP
        in_maps.append({
            "enc": np.ascontiguousarray(encoder_outputs[lo:lo + BP]),
            "hid": np.ascontiguousarray(hidden[:, lo:lo + BP, :]),
            "msk": np.ascontiguousarray(mask[lo:lo + BP]),
            "w": np.ascontiguousarray(attn_w),
            "bia": np.ascontiguousarray(attn_b),
        })
    res = run_bass_kernel_spmd(nc, in_maps, list(range(N_CORES)),
                               trace=trace, **(trace_kwargs or {}))
    full = np.concatenate([res.results[i]["out"] for i in range(N_CORES)],
                          axis=0)
    return full, res


def kernel(hidden, encoder_outputs, mask, attn_w, attn_b):
    hidden = np.asarray(hidden, dtype=np.float32)
    encoder_outputs = np.asarray(encoder_outputs, dtype=np.float32)
    mask = np.asarray(mask, dtype=np.float32)
    attn_w = np.asarray(attn_w, dtype=np.float32)
    attn_b = np.asarray(attn_b, dtype=np.float32)
    full, _ = _run(hidden, encoder_outputs, mask, attn_w, attn_b)
    return full
